# revision 28
# baseline (speedup 1.0000x reference)
"""Trainium2 Bass kernel for the 2-layer GAT + mean-pool + MLP head problem.

Strategy (8-core SPMD, single NEFF):
  - Nodes are sharded by destination across 8 cores (6250 each, padded 6272).
    Per-core local node l -> (block t = l % 49, lane p = l // 49); padded node
    table row r = core*6272 + p*49 + t so the SBUF->DRAM table write is
    contiguous per partition.
  - Per layer: each core computes an fp16 "aug" row [h | asrc | adst] (144
    cols) for its own nodes with one matmul per block (lhsT = x^T tile,
    rhs = [W | W@Asrc_bd | W@Adst_bd]); AllGather builds the full 50176-row
    gather table in every core HBM.
  - Edge phase: REAL edges (self-loops are handled densely in the epilogue,
    since they are core-local) are sorted by dst block and padded to T_b
    tiles of 128 edges per block (T_b = per-block max over cores, identical
    program on all cores).  For batches of U tiles one indirect DMA per tile
    row gathers 768B src pair-rows [h_e|h_o|asrc_e|asrc_o|pad] and a second
    gathers 256B dst pair-rows for adst — 2 descriptors per edge (SWDGE
    descriptor generation on the Q7 is the edge-phase bottleneck).
    ex = exp(max(z, 0.2z)) with z = asrc+adst; h_scaled = h*ex (broadcast
    per head); a one-hot [128e,128d] built by is_equal against an iota
    constant feeds matmul psum += onehot^T @ [h_scaled | ex], giving the
    unnormalized aggregation and the softmax denominators in one pass.
  - Block epilogue: the self-loop term exp(leaky(asrc+adst))*[h|1] is added
    from the resident local aug rows, then out = num * (1/max(s,1e-30)) per
    head, + bias, ELU; layer 1 feeds a PE transpose + matmul producing the
    next layer's aug rows; layer 2 feeds the graph-mean-pool matmul
    (device-built graph one-hot).
  - Pool partials are AllReduced (32KB), then every core runs the tiny MLP +
    log_softmax redundantly; core 0's packed [128,10] output is returned.

Host->device traffic is minimized (the axon tunnel runs at ~55MB/s): per
core we ship ONE int16 blob containing [x^T as fp8 | pk/gid/weights as f16 |
wrapped int16 gather indices | small f32 tail], all bitcast apart on device.
Iotas, identities, one-hots and parity masks are built on device.

kernel(**inputs) takes the FULL unsharded inputs and returns
(log_softmax(logits), logits) like the reference.
"""

import numpy as np

import jax

# Persistent compilation cache: the per-call XLA+NEFF pipeline is ~0.7s of
# pure recompilation of an identical module otherwise.
jax.config.update("jax_compilation_cache_dir", "/tmp/jax_bass_cache")
jax.config.update("jax_persistent_cache_min_compile_time_secs", 0)
jax.config.update("jax_persistent_cache_min_entry_size_bytes", 0)

import concourse.bass as bass
import concourse.mybir as mybir
import concourse.tile as tile
from concourse import bacc
from concourse.bass_utils import run_bass_kernel_spmd

F16 = mybir.dt.float16
F32 = mybir.dt.float32
I16 = mybir.dt.int16
F8 = mybir.dt.float8e4
AX = mybir.AluOpType

NCORES = 8
HPW = 384  # h-gather pair-row width in f16 (768B): [h_e|h_o|as_e|as_o|pad]


def gat_config(N=50000, E=800000, F=128, H=8, C=16, G=64, NCLS=10, U=24):
    NPC = N // NCORES
    BLOCKS = (NPC + 127) // 128
    NPAD = BLOCKS * 128
    return dict(N=N, E=E, F=F, H=H, C=C, G=G, NCLS=NCLS, U=U, NPC=NPC,
                BLOCKS=BLOCKS, NPAD=NPAD, TBLROWS=NCORES * NPAD, AUGW=F + 2 * H)


def _blockdiag(a, H, C):
    m = np.zeros((H * C, H), np.float32)
    for h in range(H):
        m[h * C:(h + 1) * C, h] = a[h]
    return m


def host_prep(inputs, cfg):
    """Builds per-core device input dicts + meta. Pure index/layout work."""
    N, E, F, H, C, G = cfg["N"], cfg["E"], cfg["F"], cfg["H"], cfg["C"], cfg["G"]
    NPC, BLOCKS, NPAD = cfg["NPC"], cfg["BLOCKS"], cfg["NPAD"]
    AUGW = cfg["AUGW"]
    F8np = mybir.dt.np(F8)

    x = np.asarray(inputs["x"], np.float32)
    ei = np.asarray(inputs["edge_index"], np.int64)
    batch = np.asarray(inputs["batch"], np.int64)

    W1 = np.asarray(inputs["W1"], np.float32)
    W2 = np.asarray(inputs["W2"], np.float32)
    w1aug = np.concatenate(
        [W1, W1 @ _blockdiag(np.asarray(inputs["a_src1"], np.float32), H, C),
         W1 @ _blockdiag(np.asarray(inputs["a_dst1"], np.float32), H, C)], 1)
    w2aug = np.concatenate(
        [W2, W2 @ _blockdiag(np.asarray(inputs["a_src2"], np.float32), H, C),
         W2 @ _blockdiag(np.asarray(inputs["a_dst2"], np.float32), H, C)], 1)

    # self-loops are NOT streamed: they're added densely in the epilogue
    src = ei[0]
    dst = ei[1]

    core = dst // NPC
    loc = dst - core * NPC
    t_blk = loc % BLOCKS
    p_lane = loc // BLOCKS

    def g2r(g):
        c = g // NPC
        l = g - c * NPC
        return (c * NPAD + (l // BLOCKS) * BLOCKS + (l % BLOCKS)).astype(np.int32)

    key = (core * BLOCKS + t_blk).astype(np.int64)
    order = np.argsort(key, kind="stable")
    counts = np.bincount(key, minlength=NCORES * BLOCKS)
    # per-block tile count: max over cores (same program on all cores);
    # >=1 so every block's epilogue (incl. the self-loop term) runs
    TBS = np.maximum(
        np.ceil(counts.reshape(NCORES, BLOCKS).max(0) / 128).astype(int), 1)
    NT = int(TBS.sum())
    oft = np.concatenate([[0], np.cumsum(TBS)])  # tile offset per block

    src_rows = g2r(src[order])
    dst_rows = g2r(dst[order])
    p_s = p_lane[order]

    srcR = np.zeros((NCORES, NT * 128), np.int32)
    dstR = np.zeros((NCORES, NT * 128), np.int32)
    dstloc = np.full((NCORES, NT * 128), 200.0, np.float32)
    ofs = np.concatenate([[0], np.cumsum(counts)])
    for c in range(NCORES):
        for b in range(BLOCKS):
            k = c * BLOCKS + b
            cnt = counts[k]
            sl = slice(ofs[k], ofs[k + 1])
            s0 = oft[b] * 128
            srcR[c, s0:s0 + cnt] = src_rows[sl]
            dstR[c, s0:s0 + cnt] = dst_rows[sl]
            dstloc[c, s0:s0 + cnt] = p_s[sl]

    # pk packs (dst lane | src parity | dst parity) into one f16 value:
    # pk = lane + 256*psrc + 512*pdst; pad rows keep lane=200 (no one-hot
    # match) with parity 0.  All values <= 895, exact in f16.
    pk = (dstloc + 256.0 * (srcR % 2) + 512.0 * (dstR % 2)).astype(np.float16)
    pkT = np.ascontiguousarray(
        pk.reshape(NCORES, NT, 128).transpose(0, 2, 1))  # [NC, 128, NT]

    # dma_gather index streams: int16 pair-row ids (row//2), wrapped
    # [i%16, i//16] on 16 partitions (replicated to 128 on device).
    def wrap16(stream):  # [n] -> [16, n//16] int16
        return stream.reshape(-1, 16).T.astype(np.int16)

    # only the src stream is shipped; the dst stream is derived on device
    # from dstl/pdst (dst rows are core-local: row = c*NPAD + lane*BLOCKS + b)
    idx16 = np.zeros((NCORES, 16, NT * 8), np.int16)
    for c in range(NCORES):
        idx16[c] = wrap16(srcR[c] // 2)

    # x^T per core in (t,p) column order: col t*128+p <- node c*NPC + p*BLOCKS + t
    # Shipped as fp8 e4m3 (halves the dominant wire transfer; quantization
    # error through the whole net is ~6e-3 vs the 2e-2 gate) and upcast to
    # f16 on device by a casting SWDGE DMA.
    tt = np.arange(NPAD) // 128
    pp = np.arange(NPAD) % 128
    l_of_col = pp * BLOCKS + tt
    xt = np.zeros((NCORES, F, NPAD), F8np)
    for c in range(NCORES):
        ok = l_of_col < NPC
        cols = np.where(ok, c * NPC + np.minimum(l_of_col, NPC - 1), 0)
        xr = np.where(ok[:, None], x[cols], 0.0)
        xt[c] = xr.T.astype(F8np)

    # graph id per (lane p, block t) node; 200 for pad (never matches 0..63)
    p_g, t_g = np.meshgrid(np.arange(128), np.arange(BLOCKS), indexing="ij")
    l_g = p_g * BLOCKS + t_g  # [128, BLOCKS]
    gid = np.zeros((NCORES, 128, BLOCKS), np.float16)
    for c in range(NCORES):
        okg = l_g < NPC
        gid[c] = np.where(
            okg, batch[c * NPC + np.minimum(l_g, NPC - 1)], 200.0
        ).astype(np.float16)

    cnt = np.bincount(batch, minlength=G).astype(np.float32)
    inv_cnt = (1.0 / np.maximum(cnt, 1.0)).astype(np.float32)

    b1 = np.asarray(inputs["b1"], np.float32)
    b2 = np.asarray(inputs["b2"], np.float32)
    l1b = np.asarray(inputs["lin1_b"], np.float32)
    l2b = np.asarray(inputs["lin2_b"], np.float32)
    meta = dict(cfg, NT=NT, U=min(cfg["U"], NT), TBS=[int(t) for t in TBS],
                OFT=[int(t) for t in oft],
                bias1=bool(np.any(b1 != 0)), bias2=bool(np.any(b2 != 0)),
                lbias1=bool(np.any(l1b != 0)), lbias2=bool(np.any(l2b != 0)))

    # --- per-core fp16 blob: [pkT | gid | w1aug | w2aug], padded to even
    # width so the f32 tail of the merged blob stays 4B-aligned ---
    w16 = np.concatenate([w1aug, w2aug], 1).astype(np.float16)  # [128, 2*AUGW]
    W16 = NT + BLOCKS + 2 * AUGW
    W16 += W16 % 2
    blob16 = np.zeros((NCORES, 128, W16), np.float16)
    for c in range(NCORES):
        o = 0
        blob16[c, :, o:o + NT] = pkT[c]; o += NT
        blob16[c, :, o:o + BLOCKS] = gid[c]; o += BLOCKS
        blob16[c, :, o:o + 2 * AUGW] = w16

    # --- small fp32 tail: lin1W (cols 0:16), inv_cnt (col 16, parts 0:64),
    #     lin2W (cols 17:27, parts 0:16), c*NPAD (col 27, all parts) ---
    n32 = 16 + 1 + cfg["NCLS"] + 1
    blob32 = np.zeros((128, n32), np.float32)
    blob32[:, 0:16] = np.asarray(inputs["lin1_W"], np.float32)
    blob32[0:G, 16] = inv_cnt
    blob32[0:16, 17:17 + cfg["NCLS"]] = np.asarray(inputs["lin2_W"], np.float32)

    # --- merge everything into ONE int16 array per core ---
    NTP = NT + NT % 2  # keep the f32 tail 4B-aligned
    W_ALL = NPAD // 2 + W16 + NTP + 2 * n32
    in_maps = []
    for c in range(NCORES):
        md = np.zeros((128, W_ALL), np.int16)
        o = 0
        md[:, o:o + NPAD // 2] = xt[c].view(np.int16); o += NPAD // 2
        md[:, o:o + W16] = blob16[c].view(np.int16); o += W16
        md[:, o:o + NT] = idx16[c].reshape(128, NT); o += NTP
        blob32[:, 27] = float(c * NPAD)
        md[:, o:o + 2 * n32] = blob32.view(np.int16)
        m = dict(md=md)
        if meta["bias1"]:
            m["b1rep"] = np.broadcast_to(b1.astype(np.float32), (128, F)).copy()
        if meta["bias2"]:
            m["b2rep"] = np.broadcast_to(b2.astype(np.float32), (128, F)).copy()
        if meta["lbias1"]:
            m["l1brep"] = np.broadcast_to(l1b, (G, l1b.shape[0])).copy()
        if meta["lbias2"]:
            m["l2brep"] = np.broadcast_to(l2b, (G, l2b.shape[0])).copy()
        in_maps.append(m)
    return meta, in_maps


def build_nc(meta):
    F, H, C, G, NCLS = meta["F"], meta["H"], meta["C"], meta["G"], meta["NCLS"]
    BLOCKS, NPAD, TBLROWS = meta["BLOCKS"], meta["NPAD"], meta["TBLROWS"]
    NT, U, AUGW, TBS = meta["NT"], meta["U"], meta["AUGW"], meta["TBS"]
    REPW = 2 * F + H  # matmul rhs width: [hE*exE | hO*exO | ex]
    W16 = NT + BLOCKS + 2 * AUGW
    W16 += W16 % 2
    OPK, OGID, OW1, OW2 = 0, NT, NT + BLOCKS, NT + BLOCKS + AUGW
    N32 = 18 + NCLS
    NTP = NT + NT % 2
    W_ALL = NPAD // 2 + W16 + NTP + 2 * N32
    OB16, OIDX, OB32 = NPAD // 2, NPAD // 2 + W16, NPAD // 2 + W16 + NTP
    # tile -> (block, k-within-block)
    tilemap = [(b, k) for b in range(BLOCKS) for k in range(TBS[b])]
    OFT = meta["OFT"]

    # 4 SWDGE queues: the h-gather and a-gather generate their descriptors
    # on separate queues (and consecutive chunks alternate queue pairs) so
    # the Q7 descriptor generation — the edge-phase bottleneck — overlaps
    # across both streams and adjacent chunks.
    nc = bacc.Bacc("TRN2", target_bir_lowering=False, debug=False,
                   num_devices=NCORES, num_swdge_queues=4)

    # --- I/O ---
    d_m = nc.dram_tensor("md", [128, W_ALL], I16, kind="ExternalInput")
    d_bias1 = (nc.dram_tensor("b1rep", [128, F], F32, kind="ExternalInput")
               if meta["bias1"] else None)
    d_bias2 = (nc.dram_tensor("b2rep", [128, F], F32, kind="ExternalInput")
               if meta["bias2"] else None)
    d_l1b = (nc.dram_tensor("l1brep", [G, C], F32, kind="ExternalInput")
             if meta["lbias1"] else None)
    d_l2b = (nc.dram_tensor("l2brep", [G, NCLS], F32, kind="ExternalInput")
             if meta["lbias2"] else None)
    d_out = nc.dram_tensor("out", [2 * G, NCLS], F32, kind="ExternalOutput")

    # --- internal DRAM (collectives + reformatted gather tables) ---
    aug_loc = [nc.dram_tensor(f"aug_loc{i}", [NPAD, AUGW], F16) for i in (1, 2)]
    table = [nc.dram_tensor(f"table{i}", [TBLROWS, AUGW], F16, addr_space="Shared")
             for i in (1, 2)]
    # hp: pair rows [h_e|h_o|as_e|as_o|pad] (768B); ap: pair rows with the
    # a slices at cols 48:64 (even) / 112:128 (odd) (256B)
    hp_tbl = [nc.dram_tensor(f"hp{i}", [TBLROWS // 2, HPW], F16) for i in (1, 2)]
    ap_tbl = [nc.dram_tensor(f"ap{i}", [TBLROWS // 2, 128], F16) for i in (1, 2)]
    pool_part = nc.dram_tensor("pool_part", [G, F], F32)
    pool_full = nc.dram_tensor("pool_full", [G, F], F32, addr_space="Shared")
    RG = [list(range(NCORES))]

    from contextlib import ExitStack
    with tile.TileContext(nc) as tc, ExitStack() as ctx:
        cpool = ctx.enter_context(tc.tile_pool(name="consts", bufs=1))
        gpool = ctx.enter_context(tc.tile_pool(name="gath", bufs=2))
        hpool = ctx.enter_context(tc.tile_pool(name="hsex", bufs=2))
        opool = ctx.enter_context(tc.tile_pool(name="oneh", bufs=2))
        zpool = ctx.enter_context(tc.tile_pool(name="zl", bufs=3))
        apool = ctx.enter_context(tc.tile_pool(name="adL", bufs=2))
        epool = ctx.enter_context(tc.tile_pool(name="epi", bufs=3))
        augp = ctx.enter_context(tc.tile_pool(name="augsb", bufs=2))
        psp = ctx.enter_context(tc.tile_pool(name="ps", bufs=3, space="PSUM"))
        pst = ctx.enter_context(tc.tile_pool(name="pst", bufs=2, space="PSUM"))
        psa = ctx.enter_context(tc.tile_pool(name="psa", bufs=2, space="PSUM"))
        psg = ctx.enter_context(tc.tile_pool(name="psg", bufs=1, space="PSUM"))

        # ---- load the packed blob, bitcast apart ----
        xt_sb = cpool.tile([F, NPAD], F16, tag="xt")
        nc.gpsimd.dma_start(out=xt_sb[:],
                            in_=d_m[:, 0:NPAD // 2].bitcast(F8))  # fp8->f16
        b16 = cpool.tile([128, W16], F16, tag="blob16")
        nc.sync.dma_start(out=b16[:], in_=d_m[:, OB16:OB16 + W16].bitcast(F16))
        b32 = cpool.tile([128, N32], F32, tag="blob32")
        nc.sync.dma_start(out=b32[:],
                          in_=d_m[:, OB32:OB32 + 2 * N32].bitcast(F32))
        idxr = cpool.tile([128, NT * 16], I16, tag="idxr")
        nc.sync.dma_start(
            out=idxr[0:16, 0:NT * 8].rearrange("r (j w) -> r j w", j=8),
            in_=d_m[:, OIDX:OIDX + NT].rearrange("(r j) w -> r j w", j=8))

        bias1_sb = bias2_sb = l1b_sb = l2b_sb = None
        if d_bias1 is not None:
            bias1_sb = cpool.tile([128, F], F32, tag="b1")
            nc.sync.dma_start(out=bias1_sb[:], in_=d_bias1[:, :])
        if d_bias2 is not None:
            bias2_sb = cpool.tile([128, F], F32, tag="b2")
            nc.sync.dma_start(out=bias2_sb[:], in_=d_bias2[:, :])
        if d_l1b is not None:
            l1b_sb = cpool.tile([G, C], F32, tag="l1b")
            nc.sync.dma_start(out=l1b_sb[:], in_=d_l1b[:, :])
        if d_l2b is not None:
            l2b_sb = cpool.tile([G, NCLS], F32, tag="l2b")
            nc.sync.dma_start(out=l2b_sb[:], in_=d_l2b[:, :])

        # ---- device-built constants ----
        iota_sb = cpool.tile([128, U * 128], F16, tag="iota")
        nc.gpsimd.iota(out=iota_sb[:], pattern=[[0, U], [1, 128]], base=0,
                       channel_multiplier=0,
                       allow_small_or_imprecise_dtypes=True)
        # identities via two iotas + is_equal (no negative channel mult)
        idh_sb = cpool.tile([128, 128], F16, tag="idh")
        rowh = cpool.tile([128, 128], F16, tag="rowh")
        nc.gpsimd.iota(out=idh_sb[:], pattern=[[1, 128]], base=0,
                       channel_multiplier=0,
                       allow_small_or_imprecise_dtypes=True)
        nc.gpsimd.iota(out=rowh[:], pattern=[[0, 128]], base=0,
                       channel_multiplier=1,
                       allow_small_or_imprecise_dtypes=True)
        nc.vector.tensor_tensor(out=idh_sb[:], in0=idh_sb[:], in1=rowh[:],
                                op=AX.is_equal)
        idf_sb = cpool.tile([64, 64], F32, tag="idf")
        rowf = cpool.tile([64, 64], F32, tag="rowf")
        nc.gpsimd.iota(out=idf_sb[:], pattern=[[1, 64]], base=0,
                       channel_multiplier=0,
                       allow_small_or_imprecise_dtypes=True)
        nc.gpsimd.iota(out=rowf[:], pattern=[[0, 64]], base=0,
                       channel_multiplier=1,
                       allow_small_or_imprecise_dtypes=True)
        nc.vector.tensor_tensor(out=idf_sb[:], in0=idf_sb[:], in1=rowf[:],
                                op=AX.is_equal)
        # graph one-hot: gone[p, t*G+g] = (gid[p,t] == g)
        gone_sb = cpool.tile([128, BLOCKS * G], F16, tag="gone")
        nc.gpsimd.iota(out=gone_sb[:], pattern=[[0, BLOCKS], [1, G]], base=0,
                       channel_multiplier=0,
                       allow_small_or_imprecise_dtypes=True)
        nc.vector.tensor_tensor(
            out=gone_sb[:].rearrange("p (t g) -> p t g", g=G),
            in0=gone_sb[:].rearrange("p (t g) -> p t g", g=G),
            in1=b16[:, OGID:OGID + BLOCKS].to_broadcast([128, BLOCKS, G]),
            op=AX.is_equal)

        # unpack pk -> pdst, psrc, dstl (+ src complement)
        pdst_sb = cpool.tile([128, NT], F16, tag="pdst")
        psrc_sb = cpool.tile([128, NT], F16, tag="psrc")
        dstl_sb = cpool.tile([128, NT], F16, tag="dstl")
        qsrc_sb = cpool.tile([128, NT], F16, tag="qsrc")
        nc.vector.tensor_scalar(out=pdst_sb[:], in0=b16[:, OPK:OPK + NT],
                                scalar1=512.0, scalar2=None, op0=AX.is_ge)
        nc.vector.scalar_tensor_tensor(out=dstl_sb[:], in0=pdst_sb[:],
                                       scalar=-512.0, op0=AX.mult,
                                       in1=b16[:, OPK:OPK + NT], op1=AX.add)
        nc.vector.tensor_scalar(out=psrc_sb[:], in0=dstl_sb[:],
                                scalar1=256.0, scalar2=None, op0=AX.is_ge)
        nc.vector.scalar_tensor_tensor(out=dstl_sb[:], in0=psrc_sb[:],
                                       scalar=-256.0, op0=AX.mult,
                                       in1=dstl_sb[:], op1=AX.add)
        nc.vector.tensor_scalar(out=qsrc_sb[:], in0=psrc_sb[:], scalar1=-1.0,
                                scalar2=1.0, op0=AX.mult, op1=AX.add)

        # ---- derive the dst gather stream on device ----
        # dst pair row = (c*NPAD + dstl*BLOCKS + b - pdst) / 2, computed in
        # f32 (exact), clamped for pad slots, cast to i16 and wrap-shuffled
        # into the [i%16, i//16] stream layout dma_gather expects.
        drow = cpool.tile([128, NT], F32, tag="drow")
        for b in range(BLOCKS):
            nc.vector.memset(drow[:, OFT[b]:OFT[b + 1]], float(b))
        nc.vector.scalar_tensor_tensor(out=drow[:], in0=dstl_sb[:],
                                       scalar=float(BLOCKS), op0=AX.mult,
                                       in1=drow[:], op1=AX.add)
        nc.vector.tensor_scalar(out=drow[:], in0=drow[:],
                                scalar1=b32[:, 27:28], scalar2=None,
                                op0=AX.add)
        nc.vector.scalar_tensor_tensor(out=drow[:], in0=pdst_sb[:],
                                       scalar=-1.0, op0=AX.mult,
                                       in1=drow[:], op1=AX.add)
        nc.vector.tensor_scalar(out=drow[:], in0=drow[:], scalar1=0.5,
                                scalar2=float(TBLROWS // 2 - 1), op0=AX.mult,
                                op1=AX.min)
        ph16 = cpool.tile([128, NT], I16, tag="ph16")
        nc.vector.tensor_copy(out=ph16[:], in_=drow[:])
        wrapv = idxr[0:16, NT * 8:NT * 16].rearrange("r (u q) -> r u q", q=8)
        for q in range(8):
            nc.sync.dma_start(
                out=wrapv[:, :, q:q + 1],
                in_=ph16[q * 16:(q + 1) * 16, :].rearrange(
                    "p (u one) -> p u one", one=1))
        # replicate 16 -> 128 partitions (dma_gather wants the stream on
        # every 16-partition group)
        nc.sync.dma_start(out=idxr[16:32, :], in_=idxr[0:16, :])
        nc.sync.dma_start(out=idxr[32:64, :], in_=idxr[0:32, :])
        nc.sync.dma_start(out=idxr[64:128, :], in_=idxr[0:64, :])

        def build_aug_from_xt(woff):
            """aug rows for own nodes from resident x^T; returns sbuf tile."""
            aug_sb = augp.tile([128, BLOCKS * AUGW], F16, tag="augsb")
            for t in range(BLOCKS):
                ps = psa.tile([128, AUGW], F32, tag="psaug")
                nc.tensor.matmul(out=ps[:], lhsT=xt_sb[:, t * 128:(t + 1) * 128],
                                 rhs=b16[:, woff:woff + AUGW],
                                 start=True, stop=True)
                nc.vector.tensor_copy(out=aug_sb[:, t * AUGW:(t + 1) * AUGW],
                                      in_=ps[:])
            return aug_sb

        def publish_table(aug_sb, which):
            dst = aug_loc[which]
            # DRAM rows r = p*BLOCKS + t  <=> view [(p t), f] -> [p, (t f)]
            nc.sync.dma_start(
                out=dst[:, :].rearrange("(p t) f -> p (t f)", t=BLOCKS),
                in_=aug_sb[:])
            nc.gpsimd.collective_compute(
                "AllGather", AX.bypass, replica_groups=RG,
                ins=[dst[:, :].opt()], outs=[table[which][:, :].opt()])
            # reformat into pair-row gather tables (DRAM->DRAM)
            t3 = table[which][:, :].rearrange("(g two) f -> g two f", two=2)
            nc.sync.dma_start(
                out=hp_tbl[which][:, 0:2 * F].rearrange(
                    "g (two f) -> g two f", two=2),
                in_=t3[:, :, 0:F])
            nc.sync.dma_start(
                out=hp_tbl[which][:, 2 * F:2 * F + 2 * H].rearrange(
                    "g (two a) -> g two a", two=2),
                in_=t3[:, :, F:F + H])
            # full 128-col rows (finite pad): cols 48:64 = a_even,
            # cols 112:128 = a_odd; 0:48/64:112 are h-tail junk
            nc.sync.dma_start(
                out=ap_tbl[which][:, :].rearrange("g (two j) -> g two j", two=2),
                in_=t3[:, :, F - 48:F + 2 * H])

        def elu_inplace(v_sb, width, out_tile):
            """out_tile(fp16) = elu(v_sb) = max(v,0) + min(exp(v)-1, 0)."""
            t_sb = epool.tile([128, width], F32, tag="elu_t")
            nc.scalar.activation(out=t_sb[:], in_=v_sb[:],
                                 func=mybir.ActivationFunctionType.Exp)
            nc.vector.tensor_scalar(out=t_sb[:], in0=t_sb[:], scalar1=1.0,
                                    scalar2=0.0, op0=AX.subtract, op1=AX.min)
            nc.vector.scalar_tensor_tensor(out=out_tile[:], in0=v_sb[:],
                                           scalar=0.0, op0=AX.max,
                                           in1=t_sb[:], op1=AX.add)

        def edge_phase(layer, aug_sb):
            """layer 0: consumes table[0], produces aug tile for table[1].
               layer 1: consumes table[1], accumulates pool psum.  aug_sb is
               the CURRENT layer's local aug tile (for the dense self-loop
               term).  Returns next aug tile (layer 0) or pool psum."""
            bias_sb = (bias1_sb, bias2_sb)[layer]
            if layer == 0:
                out_aug = augp.tile([128, BLOCKS * AUGW], F16, tag="augsb")
            else:
                pool_ps = psg.tile([G, F], F32, tag="poolps")

            hp, ap = hp_tbl[layer], ap_tbl[layer]
            nbatch = (NT + U - 1) // U
            ps_cur = None
            for bi in range(nbatch):
                u0 = bi * U
                ub = min(U, NT - u0)
                # bulk gathers: [h|asrc] pair-rows by src//2 (768B) and a
                # pair-rows by dst//2 (256B) — 2 descriptors per edge
                ghp = gpool.tile([128, U * HPW], F16, tag="g")
                nc.gpsimd.dma_gather(
                    out_ap=ghp[:, :ub * HPW].rearrange(
                        "p (u f) -> p u f", f=HPW),
                    in_ap=hp[:, :], idxs_ap=idxr[:, u0 * 8:(u0 + ub) * 8],
                    num_idxs=ub * 128, num_idxs_reg=ub * 128, elem_size=HPW,
                    single_packet=False, queue_num=(bi % 2) * 2)
                gap = apool.tile([128, U * 128], F16, tag="gap")
                nc.gpsimd.dma_gather(
                    out_ap=gap[:, :ub * 128].rearrange(
                        "p (u f) -> p u f", f=128),
                    in_ap=ap[:, :],
                    idxs_ap=idxr[:, NT * 8 + u0 * 8:NT * 8 + (u0 + ub) * 8],
                    num_idxs=ub * 128, num_idxs_reg=ub * 128,
                    elem_size=128, single_packet=False,
                    queue_num=1 + (bi % 2) * 2)
                g3 = ghp[:, :ub * HPW].rearrange("p (u f) -> p u f", f=HPW)
                ga = gap[:, :ub * 128].rearrange("p (u f) -> p u f", f=128)

                # z = asrc[src] + adst[dst] with parity selection:
                #   asrc = ae + psrc*(ao-ae); adst = be + pdst*(bo-be)
                zl = zpool.tile([128, U * H], F16, tag="zl")
                tsel = zpool.tile([128, U * H], F16, tag="tsel")
                psB = psrc_sb[:, u0:u0 + ub].to_broadcast([128, ub, H])
                pdB = pdst_sb[:, u0:u0 + ub].to_broadcast([128, ub, H])
                t3 = tsel[:, :ub * H].rearrange("p (u h) -> p u h", h=H)
                z3 = zl[:, :ub * H].rearrange("p (u h) -> p u h", h=H)
                nc.vector.tensor_tensor(out=t3, in0=g3[:, :, 2 * F + H:2 * F + 2 * H],
                                        in1=g3[:, :, 2 * F:2 * F + H],
                                        op=AX.subtract)
                nc.vector.tensor_tensor(out=t3, in0=t3, in1=psB, op=AX.mult)
                nc.vector.tensor_tensor(out=z3, in0=t3,
                                        in1=g3[:, :, 2 * F:2 * F + H], op=AX.add)
                nc.vector.tensor_tensor(out=t3, in0=ga[:, :, 120:128],
                                        in1=ga[:, :, 56:64], op=AX.subtract)
                nc.vector.tensor_tensor(out=t3, in0=t3, in1=pdB, op=AX.mult)
                nc.vector.tensor_tensor(out=z3, in0=z3, in1=t3, op=AX.add)
                nc.vector.tensor_tensor(out=z3, in0=z3,
                                        in1=ga[:, :, 56:64], op=AX.add)
                zv = zl[:, :ub * H]
                nc.vector.scalar_tensor_tensor(
                    out=zv, in0=zv, scalar=0.2, op0=AX.mult, in1=zv, op1=AX.max)

                he = hpool.tile([128, U * REPW], F16, tag="he")
                he3 = he[:, :ub * REPW].rearrange("p (u f) -> p u f", f=REPW)
                nc.scalar.activation(
                    out=he3[:, :, 2 * F:2 * F + H],
                    in_=zl[:, :ub * H].rearrange("p (u h) -> p u h", h=H),
                    func=mybir.ActivationFunctionType.Exp)
                # parity-masked ex, folded into the h scaling: the even half is
                # scaled by ex*(1-psrc), the odd half by ex*psrc, so the wrong
                # parity contributes zero and the psum halves sum to the answer
                exE = zpool.tile([128, U * H], F16, tag="exE")
                exO = zpool.tile([128, U * H], F16, tag="exO")
                eE3 = exE[:, :ub * H].rearrange("p (u h) -> p u h", h=H)
                eO3 = exO[:, :ub * H].rearrange("p (u h) -> p u h", h=H)
                nc.vector.tensor_tensor(
                    out=eE3, in0=he3[:, :, 2 * F:2 * F + H],
                    in1=qsrc_sb[:, u0:u0 + ub].to_broadcast([128, ub, H]),
                    op=AX.mult)
                nc.vector.tensor_tensor(
                    out=eO3, in0=he3[:, :, 2 * F:2 * F + H], in1=psB,
                    op=AX.mult)
                nc.vector.tensor_tensor(
                    out=he3[:, :, 0:F].rearrange("p u (h c) -> p u h c", c=C),
                    in0=g3[:, :, 0:F].rearrange("p u (h c) -> p u h c", c=C),
                    in1=eE3.to_broadcast([128, ub, H, C]), op=AX.mult)
                nc.vector.tensor_tensor(
                    out=he3[:, :, F:2 * F].rearrange("p u (h c) -> p u h c", c=C),
                    in0=g3[:, :, F:2 * F].rearrange("p u (h c) -> p u h c", c=C),
                    in1=eO3.to_broadcast([128, ub, H, C]), op=AX.mult)

                oh = opool.tile([128, U * 128], F16, tag="oh")
                nc.vector.tensor_tensor(
                    out=oh[:, :ub * 128].rearrange("p (u j) -> p u j", j=128),
                    in0=iota_sb[:, :ub * 128].rearrange("p (u j) -> p u j", j=128),
                    in1=dstl_sb[:, u0:u0 + ub].to_broadcast([128, ub, 128]),
                    op=AX.is_equal)

                for u in range(ub):
                    t = u0 + u
                    b, k = tilemap[t]
                    if k == 0:
                        ps_cur = psp.tile([128, REPW], F32, tag="psblk")
                    nc.tensor.matmul(
                        out=ps_cur[:], lhsT=oh[:, u * 128:(u + 1) * 128],
                        rhs=he[:, u * REPW:(u + 1) * REPW],
                        start=(k == 0), stop=(k == TBS[b] - 1))
                    if k == TBS[b] - 1:
                        # ---- block epilogue ----
                        # dense self-loop term from the local aug rows:
                        # ex_self = exp(leaky(asrc+adst)), s += ex_self,
                        # num += h_local * ex_self
                        zs = epool.tile([128, H], F16, tag="zs")
                        nc.vector.tensor_tensor(
                            out=zs[:], in0=aug_sb[:, b * AUGW + F:b * AUGW + F + H],
                            in1=aug_sb[:, b * AUGW + F + H:b * AUGW + F + 2 * H],
                            op=AX.add)
                        nc.vector.scalar_tensor_tensor(
                            out=zs[:], in0=zs[:], scalar=0.2, op0=AX.mult,
                            in1=zs[:], op1=AX.max)
                        exs = epool.tile([128, H], F32, tag="exs")
                        nc.scalar.activation(out=exs[:], in_=zs[:],
                                             func=mybir.ActivationFunctionType.Exp)
                        s_sb = epool.tile([128, H], F32, tag="s")
                        nc.vector.tensor_tensor(out=s_sb[:], in0=exs[:],
                                                in1=ps_cur[:, 2 * F:2 * F + H],
                                                op=AX.add)
                        nc.vector.tensor_scalar(out=s_sb[:], in0=s_sb[:],
                                                scalar1=1e-30, scalar2=None,
                                                op0=AX.max)
                        r_sb = epool.tile([128, H], F32, tag="r")
                        nc.vector.reciprocal(out=r_sb[:], in_=s_sb[:])
                        hs_sb = epool.tile([128, F], F32, tag="hs")
                        nc.vector.tensor_tensor(
                            out=hs_sb[:].rearrange("p (h c) -> p h c", c=C),
                            in0=aug_sb[:, b * AUGW:b * AUGW + F].rearrange(
                                "p (h c) -> p h c", c=C),
                            in1=exs[:].to_broadcast([128, H, C]), op=AX.mult)
                        hc_sb = epool.tile([128, F], F32, tag="hc")
                        nc.vector.tensor_tensor(out=hc_sb[:], in0=hs_sb[:],
                                                in1=ps_cur[:, 0:F], op=AX.add)
                        nc.vector.tensor_tensor(out=hc_sb[:], in0=hc_sb[:],
                                                in1=ps_cur[:, F:2 * F], op=AX.add)
                        v_sb = epool.tile([128, F], F32, tag="v")
                        nc.vector.tensor_tensor(
                            out=v_sb[:].rearrange("p (h c) -> p h c", c=C),
                            in0=hc_sb[:].rearrange("p (h c) -> p h c", c=C),
                            in1=r_sb[:].to_broadcast([128, H, C]), op=AX.mult)
                        if bias_sb is not None:
                            nc.vector.tensor_tensor(out=v_sb[:], in0=v_sb[:],
                                                    in1=bias_sb[:], op=AX.add)
                        eo = epool.tile([128, F], F16, tag="eo")
                        elu_inplace(v_sb, F, eo)
                        if layer == 0:
                            trp = pst.tile([128, 128], F16, tag="trps")
                            nc.tensor.transpose(out=trp[:], in_=eo[:],
                                                identity=idh_sb[:])
                            trs = epool.tile([128, 128], F16, tag="trsb")
                            nc.vector.tensor_copy(out=trs[:], in_=trp[:])
                            ap2 = psa.tile([128, AUGW], F32, tag="psaug")
                            nc.tensor.matmul(out=ap2[:], lhsT=trs[:],
                                             rhs=b16[:, OW2:OW2 + AUGW],
                                             start=True, stop=True)
                            nc.vector.tensor_copy(
                                out=out_aug[:, b * AUGW:(b + 1) * AUGW],
                                in_=ap2[:])
                        else:
                            nc.tensor.matmul(
                                out=pool_ps[:],
                                lhsT=gone_sb[:, b * G:(b + 1) * G],
                                rhs=eo[:], start=(b == 0), stop=(b == BLOCKS - 1))
            return out_aug if layer == 0 else pool_ps

        # ---------------- pipeline ----------------
        aug1_sb = build_aug_from_xt(OW1)
        publish_table(aug1_sb, 0)
        aug2_sb = edge_phase(0, aug1_sb)
        publish_table(aug2_sb, 1)
        pool_ps = edge_phase(1, aug2_sb)

        # pooling allreduce
        psum_sb = epool.tile([G, F], F32, tag="poolsb")
        nc.vector.tensor_copy(out=psum_sb[:], in_=pool_ps[:])
        nc.sync.dma_start(out=pool_part[:, :], in_=psum_sb[:])
        nc.gpsimd.collective_compute(
            "AllReduce", AX.add, replica_groups=RG,
            ins=[pool_part[:, :].opt()], outs=[pool_full[:, :].opt()])
        hg_sb = epool.tile([G, F], F32, tag="hg")
        nc.sync.dma_start(out=hg_sb[:], in_=pool_full[:, :])
        nc.vector.tensor_scalar(out=hg_sb[:], in0=hg_sb[:],
                                scalar1=b32[0:G, 16:17], scalar2=None,
                                op0=AX.mult)

        # MLP: z1 = elu(hg @ lin1W + b); logits = z1 @ lin2W + b
        hgT_ps = pst.tile([F, G], F32, tag="trps")
        nc.tensor.transpose(out=hgT_ps[:], in_=hg_sb[:], identity=idf_sb[:G, :G])
        hgT_sb = epool.tile([F, G], F32, tag="hgTs")
        nc.vector.tensor_copy(out=hgT_sb[:], in_=hgT_ps[:])
        z1_ps = psa.tile([G, C], F32, tag="psaug")
        nc.tensor.matmul(out=z1_ps[:], lhsT=hgT_sb[:], rhs=b32[:, 0:16],
                         start=True, stop=True)
        z1_sb = epool.tile([G, C], F32, tag="z1s")
        if l1b_sb is not None:
            nc.vector.tensor_tensor(out=z1_sb[:], in0=z1_ps[:], in1=l1b_sb[:],
                                    op=AX.add)
        else:
            nc.vector.tensor_copy(out=z1_sb[:], in_=z1_ps[:])
        z1e_sb = epool.tile([G, C], F32, tag="z1e")
        t1 = epool.tile([G, C], F32, tag="t1")
        nc.scalar.activation(out=t1[:], in_=z1_sb[:],
                             func=mybir.ActivationFunctionType.Exp)
        nc.vector.tensor_scalar(out=t1[:], in0=t1[:], scalar1=1.0, scalar2=0.0,
                                op0=AX.subtract, op1=AX.min)
        nc.vector.scalar_tensor_tensor(out=z1e_sb[:], in0=z1_sb[:], scalar=0.0,
                                       op0=AX.max, in1=t1[:], op1=AX.add)
        z1T_ps = pst.tile([C, G], F32, tag="trps")
        nc.tensor.transpose(out=z1T_ps[:], in_=z1e_sb[:], identity=idf_sb[:G, :G])
        z1T_sb = epool.tile([C, G], F32, tag="z1Ts")
        nc.vector.tensor_copy(out=z1T_sb[:], in_=z1T_ps[:])
        lg_ps = psa.tile([G, NCLS], F32, tag="psaug")
        nc.tensor.matmul(out=lg_ps[:], lhsT=z1T_sb[:], rhs=b32[0:16, 17:17 + NCLS],
                         start=True, stop=True)
        lg_sb = epool.tile([G, NCLS], F32, tag="lgs")
        if l2b_sb is not None:
            nc.vector.tensor_tensor(out=lg_sb[:], in0=lg_ps[:], in1=l2b_sb[:],
                                    op=AX.add)
        else:
            nc.vector.tensor_copy(out=lg_sb[:], in_=lg_ps[:])

        # log_softmax
        m_sb = epool.tile([G, 1], F32, tag="m")
        nc.vector.tensor_reduce(out=m_sb[:], in_=lg_sb[:],
                                axis=mybir.AxisListType.X, op=AX.max)
        nm_sb = epool.tile([G, 1], F32, tag="nm")
        nc.vector.tensor_scalar(out=nm_sb[:], in0=m_sb[:], scalar1=-1.0,
                                scalar2=None, op0=AX.mult)
        e_sb = epool.tile([G, NCLS], F32, tag="esm")
        ss_sb = epool.tile([G, 1], F32, tag="ss")
        nc.scalar.activation(out=e_sb[:], in_=lg_sb[:],
                             func=mybir.ActivationFunctionType.Exp,
                             bias=nm_sb[:, 0:1], accum_out=ss_sb[:, 0:1])
        ls_sb = epool.tile([G, 1], F32, tag="ls")
        nc.scalar.activation(out=ls_sb[:], in_=ss_sb[:],
                             func=mybir.ActivationFunctionType.Ln)
        lsm_sb = epool.tile([G, NCLS], F32, tag="lsm")
        nc.vector.tensor_scalar(out=lsm_sb[:], in0=lg_sb[:],
                                scalar1=m_sb[:, 0:1], scalar2=ls_sb[:, 0:1],
                                op0=AX.subtract, op1=AX.subtract)

        nc.sync.dma_start(out=d_out[0:G, :], in_=lsm_sb[:])
        nc.sync.dma_start(out=d_out[G:2 * G, :], in_=lg_sb[:])

    nc.compile()  # bacc register allocation / DCE / act-table loads
    # The module is immutable from here on; memoize its (deterministic)
    # serialization so each run_bass_kernel_spmd call skips the ~0.1s
    # re-serialization in the jit lowering path.
    _json = nc.to_json_bytes()
    nc.to_json_bytes = lambda: _json
    return nc


def run_gat(inputs, cfg, trace=False):
    meta, in_maps = host_prep(inputs, cfg)
    nc = build_nc(meta)
    res = run_bass_kernel_spmd(nc, in_maps, core_ids=list(range(NCORES)),
                               trace=trace)
    G, NCLS = cfg["G"], cfg["NCLS"]
    out = res.results[0]["out"]
    return (out[0:G, :], out[G:2 * G, :]), res


def kernel(**inputs):
    (lsm, logits), _ = run_gat(inputs, gat_config())
    return lsm.astype(np.float32), logits.astype(np.float32)


# revision 29
# speedup vs baseline: 1.5231x; 1.5231x over previous
"""Trainium2 Bass kernel for the 2-layer GAT + mean-pool + MLP head problem.

Strategy (8-core SPMD, single NEFF):
  - Nodes are sharded by destination across 8 cores (6250 each, padded 6272).
    Per-core local node l -> (block t = l % 49, lane p = l // 49); padded node
    table row r = core*6272 + p*49 + t so the SBUF->DRAM table write is
    contiguous per partition.
  - Per layer: each core computes an fp16 "aug" row [h | asrc | adst] (144
    cols) for its own nodes with one matmul per block (lhsT = x^T tile,
    rhs = [W | W@Asrc_bd | W@Adst_bd]); AllGather builds the full 50176-row
    gather table in every core HBM.
  - Edge phase: REAL edges (self-loops are handled densely in the epilogue,
    since they are core-local) are sorted by dst block and padded to T_b
    tiles of 128 edges per block (T_b = per-block max over cores, identical
    program on all cores).  For batches of U tiles one indirect DMA per tile
    row gathers 768B src pair-rows [h_e|h_o|asrc_e|asrc_o|pad] and a second
    gathers 256B dst pair-rows for adst — 2 descriptors per edge (SWDGE
    descriptor generation on the Q7 is the edge-phase bottleneck).
    ex = exp(max(z, 0.2z)) with z = asrc+adst; h_scaled = h*ex (broadcast
    per head); a one-hot [128e,128d] built by is_equal against an iota
    constant feeds matmul psum += onehot^T @ [h_scaled | ex], giving the
    unnormalized aggregation and the softmax denominators in one pass.
  - Block epilogue: the self-loop term exp(leaky(asrc+adst))*[h|1] is added
    from the resident local aug rows, then out = num * (1/max(s,1e-30)) per
    head, + bias, ELU; layer 1 feeds a PE transpose + matmul producing the
    next layer's aug rows; layer 2 feeds the graph-mean-pool matmul
    (device-built graph one-hot).
  - Pool partials are AllReduced (32KB), then every core runs the tiny MLP +
    log_softmax redundantly; core 0's packed [128,10] output is returned.

Host->device traffic is minimized (the axon tunnel runs at ~55MB/s): per
core we ship ONE int16 blob containing [x^T as fp8 | pk/gid/weights as f16 |
wrapped int16 gather indices | small f32 tail], all bitcast apart on device.
Iotas, identities, one-hots and parity masks are built on device.

kernel(**inputs) takes the FULL unsharded inputs and returns
(log_softmax(logits), logits) like the reference.
"""

import numpy as np

import jax

# Persistent compilation cache: the per-call XLA+NEFF pipeline is ~0.7s of
# pure recompilation of an identical module otherwise.
jax.config.update("jax_compilation_cache_dir", "/tmp/jax_bass_cache")
jax.config.update("jax_persistent_cache_min_compile_time_secs", 0)
jax.config.update("jax_persistent_cache_min_entry_size_bytes", 0)

import concourse.bass as bass
import concourse.mybir as mybir
import concourse.tile as tile
from concourse import bacc
from concourse.bass_utils import run_bass_kernel_spmd

F16 = mybir.dt.float16
F32 = mybir.dt.float32
I16 = mybir.dt.int16
F8 = mybir.dt.float8e4
AX = mybir.AluOpType

NCORES = 8
HPW = 384  # h-gather pair-row width in f16 (768B): [h_e|h_o|as_e|as_o|pad]


def gat_config(N=50000, E=800000, F=128, H=8, C=16, G=64, NCLS=10, U=24):
    NPC = N // NCORES
    BLOCKS = (NPC + 127) // 128
    NPAD = BLOCKS * 128
    return dict(N=N, E=E, F=F, H=H, C=C, G=G, NCLS=NCLS, U=U, NPC=NPC,
                BLOCKS=BLOCKS, NPAD=NPAD, TBLROWS=NCORES * NPAD, AUGW=F + 2 * H)


def _blockdiag(a, H, C):
    m = np.zeros((H * C, H), np.float32)
    for h in range(H):
        m[h * C:(h + 1) * C, h] = a[h]
    return m


def host_prep(inputs, cfg):
    """Builds per-core device input dicts + meta. Pure index/layout work."""
    N, E, F, H, C, G = cfg["N"], cfg["E"], cfg["F"], cfg["H"], cfg["C"], cfg["G"]
    NPC, BLOCKS, NPAD = cfg["NPC"], cfg["BLOCKS"], cfg["NPAD"]
    AUGW = cfg["AUGW"]
    F8np = mybir.dt.np(F8)

    x = np.asarray(inputs["x"], np.float32)
    ei = np.asarray(inputs["edge_index"], np.int64)
    batch = np.asarray(inputs["batch"], np.int64)

    W1 = np.asarray(inputs["W1"], np.float32)
    W2 = np.asarray(inputs["W2"], np.float32)
    w1aug = np.concatenate(
        [W1, W1 @ _blockdiag(np.asarray(inputs["a_src1"], np.float32), H, C),
         W1 @ _blockdiag(np.asarray(inputs["a_dst1"], np.float32), H, C)], 1)
    w2aug = np.concatenate(
        [W2, W2 @ _blockdiag(np.asarray(inputs["a_src2"], np.float32), H, C),
         W2 @ _blockdiag(np.asarray(inputs["a_dst2"], np.float32), H, C)], 1)

    # self-loops are NOT streamed: they're added densely in the epilogue
    src = ei[0]
    dst = ei[1]

    core = dst // NPC
    loc = dst - core * NPC
    t_blk = loc % BLOCKS
    p_lane = loc // BLOCKS

    def g2r(g):
        c = g // NPC
        l = g - c * NPC
        return (c * NPAD + (l // BLOCKS) * BLOCKS + (l % BLOCKS)).astype(np.int32)

    key = (core * BLOCKS + t_blk).astype(np.int64)
    order = np.argsort(key, kind="stable")
    counts = np.bincount(key, minlength=NCORES * BLOCKS)
    # per-block tile count: max over cores (same program on all cores);
    # >=1 so every block's epilogue (incl. the self-loop term) runs
    TBS = np.maximum(
        np.ceil(counts.reshape(NCORES, BLOCKS).max(0) / 128).astype(int), 1)
    NT = int(TBS.sum())
    oft = np.concatenate([[0], np.cumsum(TBS)])  # tile offset per block

    src_rows = g2r(src[order])
    dst_rows = g2r(dst[order])
    p_s = p_lane[order]

    srcR = np.zeros((NCORES, NT * 128), np.int32)
    dstR = np.zeros((NCORES, NT * 128), np.int32)
    dstloc = np.full((NCORES, NT * 128), 200.0, np.float32)
    ofs = np.concatenate([[0], np.cumsum(counts)])
    for c in range(NCORES):
        for b in range(BLOCKS):
            k = c * BLOCKS + b
            cnt = counts[k]
            sl = slice(ofs[k], ofs[k + 1])
            s0 = oft[b] * 128
            srcR[c, s0:s0 + cnt] = src_rows[sl]
            dstR[c, s0:s0 + cnt] = dst_rows[sl]
            dstloc[c, s0:s0 + cnt] = p_s[sl]

    # pk packs (dst lane | src parity | dst parity) into one f16 value:
    # pk = lane + 256*psrc + 512*pdst; pad rows keep lane=200 (no one-hot
    # match) with parity 0.  All values <= 895, exact in f16.
    pk = (dstloc + 256.0 * (srcR % 2) + 512.0 * (dstR % 2)).astype(np.float16)
    pkT = np.ascontiguousarray(
        pk.reshape(NCORES, NT, 128).transpose(0, 2, 1))  # [NC, 128, NT]

    # dma_gather index streams: int16 pair-row ids (row//2), wrapped
    # [i%16, i//16] on 16 partitions (replicated to 128 on device).
    def wrap16(stream):  # [n] -> [16, n//16] int16
        return stream.reshape(-1, 16).T.astype(np.int16)

    # only the src stream is shipped; the dst stream is derived on device
    # from dstl/pdst (dst rows are core-local: row = c*NPAD + lane*BLOCKS + b)
    idx16 = np.zeros((NCORES, 16, NT * 8), np.int16)
    for c in range(NCORES):
        idx16[c] = wrap16(srcR[c] // 2)

    # x^T per core in (t,p) column order: col t*128+p <- node c*NPC + p*BLOCKS + t
    # Shipped as fp8 e4m3 (halves the dominant wire transfer; quantization
    # error through the whole net is ~6e-3 vs the 2e-2 gate) and upcast to
    # f16 on device by a casting SWDGE DMA.
    tt = np.arange(NPAD) // 128
    pp = np.arange(NPAD) % 128
    l_of_col = pp * BLOCKS + tt
    xt = np.zeros((NCORES, F, NPAD), F8np)
    for c in range(NCORES):
        ok = l_of_col < NPC
        cols = np.where(ok, c * NPC + np.minimum(l_of_col, NPC - 1), 0)
        xr = np.where(ok[:, None], x[cols], 0.0)
        xt[c] = xr.T.astype(F8np)

    # graph id per (lane p, block t) node; 200 for pad (never matches 0..63)
    p_g, t_g = np.meshgrid(np.arange(128), np.arange(BLOCKS), indexing="ij")
    l_g = p_g * BLOCKS + t_g  # [128, BLOCKS]
    gid = np.zeros((NCORES, 128, BLOCKS), np.float16)
    for c in range(NCORES):
        okg = l_g < NPC
        gid[c] = np.where(
            okg, batch[c * NPC + np.minimum(l_g, NPC - 1)], 200.0
        ).astype(np.float16)

    cnt = np.bincount(batch, minlength=G).astype(np.float32)
    inv_cnt = (1.0 / np.maximum(cnt, 1.0)).astype(np.float32)

    b1 = np.asarray(inputs["b1"], np.float32)
    b2 = np.asarray(inputs["b2"], np.float32)
    l1b = np.asarray(inputs["lin1_b"], np.float32)
    l2b = np.asarray(inputs["lin2_b"], np.float32)
    meta = dict(cfg, NT=NT, U=min(cfg["U"], NT), TBS=[int(t) for t in TBS],
                OFT=[int(t) for t in oft],
                bias1=bool(np.any(b1 != 0)), bias2=bool(np.any(b2 != 0)),
                lbias1=bool(np.any(l1b != 0)), lbias2=bool(np.any(l2b != 0)))

    # --- per-core fp16 blob: [pkT | gid | w1aug | w2aug], padded to even
    # width so the f32 tail of the merged blob stays 4B-aligned ---
    w16 = np.concatenate([w1aug, w2aug], 1).astype(np.float16)  # [128, 2*AUGW]
    W16 = NT + BLOCKS + 2 * AUGW
    W16 += W16 % 2
    blob16 = np.zeros((NCORES, 128, W16), np.float16)
    for c in range(NCORES):
        o = 0
        blob16[c, :, o:o + NT] = pkT[c]; o += NT
        blob16[c, :, o:o + BLOCKS] = gid[c]; o += BLOCKS
        blob16[c, :, o:o + 2 * AUGW] = w16

    # --- small fp32 tail: lin1W (cols 0:16), inv_cnt (col 16, parts 0:64),
    #     lin2W (cols 17:27, parts 0:16), c*NPAD (col 27, all parts) ---
    n32 = 16 + 1 + cfg["NCLS"] + 1
    blob32 = np.zeros((128, n32), np.float32)
    blob32[:, 0:16] = np.asarray(inputs["lin1_W"], np.float32)
    blob32[0:G, 16] = inv_cnt
    blob32[0:16, 17:17 + cfg["NCLS"]] = np.asarray(inputs["lin2_W"], np.float32)

    # --- merge everything into ONE int16 array per core ---
    NTP = NT + NT % 2  # keep the f32 tail 4B-aligned
    W_ALL = NPAD // 2 + W16 + NTP + 2 * n32
    in_maps = []
    for c in range(NCORES):
        md = np.zeros((128, W_ALL), np.int16)
        o = 0
        md[:, o:o + NPAD // 2] = xt[c].view(np.int16); o += NPAD // 2
        md[:, o:o + W16] = blob16[c].view(np.int16); o += W16
        md[:, o:o + NT] = idx16[c].reshape(128, NT); o += NTP
        blob32[:, 27] = float(c * NPAD)
        md[:, o:o + 2 * n32] = blob32.view(np.int16)
        m = dict(md=md)
        if meta["bias1"]:
            m["b1rep"] = np.broadcast_to(b1.astype(np.float32), (128, F)).copy()
        if meta["bias2"]:
            m["b2rep"] = np.broadcast_to(b2.astype(np.float32), (128, F)).copy()
        if meta["lbias1"]:
            m["l1brep"] = np.broadcast_to(l1b, (G, l1b.shape[0])).copy()
        if meta["lbias2"]:
            m["l2brep"] = np.broadcast_to(l2b, (G, l2b.shape[0])).copy()
        in_maps.append(m)
    return meta, in_maps


def build_nc(meta):
    F, H, C, G, NCLS = meta["F"], meta["H"], meta["C"], meta["G"], meta["NCLS"]
    BLOCKS, NPAD, TBLROWS = meta["BLOCKS"], meta["NPAD"], meta["TBLROWS"]
    NT, U, AUGW, TBS = meta["NT"], meta["U"], meta["AUGW"], meta["TBS"]
    REPW = 2 * F + H  # matmul rhs width: [hE*exE | hO*exO | ex]
    W16 = NT + BLOCKS + 2 * AUGW
    W16 += W16 % 2
    OPK, OGID, OW1, OW2 = 0, NT, NT + BLOCKS, NT + BLOCKS + AUGW
    N32 = 18 + NCLS
    NTP = NT + NT % 2
    W_ALL = NPAD // 2 + W16 + NTP + 2 * N32
    OB16, OIDX, OB32 = NPAD // 2, NPAD // 2 + W16, NPAD // 2 + W16 + NTP
    # tile -> (block, k-within-block)
    tilemap = [(b, k) for b in range(BLOCKS) for k in range(TBS[b])]
    OFT = meta["OFT"]

    # 2 SWDGE queues: the h-gather and a-gather generate their descriptors
    # on separate queues so the Q7 descriptor generation (the edge-phase
    # bottleneck) for the two streams can overlap.
    nc = bacc.Bacc("TRN2", target_bir_lowering=False, debug=False,
                   num_devices=NCORES, num_swdge_queues=2)

    # --- I/O ---
    d_m = nc.dram_tensor("md", [128, W_ALL], I16, kind="ExternalInput")
    d_bias1 = (nc.dram_tensor("b1rep", [128, F], F32, kind="ExternalInput")
               if meta["bias1"] else None)
    d_bias2 = (nc.dram_tensor("b2rep", [128, F], F32, kind="ExternalInput")
               if meta["bias2"] else None)
    d_l1b = (nc.dram_tensor("l1brep", [G, C], F32, kind="ExternalInput")
             if meta["lbias1"] else None)
    d_l2b = (nc.dram_tensor("l2brep", [G, NCLS], F32, kind="ExternalInput")
             if meta["lbias2"] else None)
    d_out = nc.dram_tensor("out", [2 * G, NCLS], F32, kind="ExternalOutput")

    # --- internal DRAM (collectives + reformatted gather tables) ---
    aug_loc = [nc.dram_tensor(f"aug_loc{i}", [NPAD, AUGW], F16) for i in (1, 2)]
    table = [nc.dram_tensor(f"table{i}", [TBLROWS, AUGW], F16, addr_space="Shared")
             for i in (1, 2)]
    # hp: pair rows [h_e|h_o|as_e|as_o|pad] (768B); ap: pair rows with the
    # a slices at cols 48:64 (even) / 112:128 (odd) (256B)
    hp_tbl = [nc.dram_tensor(f"hp{i}", [TBLROWS // 2, HPW], F16) for i in (1, 2)]
    ap_tbl = [nc.dram_tensor(f"ap{i}", [TBLROWS // 2, 128], F16) for i in (1, 2)]
    pool_part = nc.dram_tensor("pool_part", [G, F], F32)
    pool_full = nc.dram_tensor("pool_full", [G, F], F32, addr_space="Shared")
    RG = [list(range(NCORES))]

    from contextlib import ExitStack
    with tile.TileContext(nc) as tc, ExitStack() as ctx:
        cpool = ctx.enter_context(tc.tile_pool(name="consts", bufs=1))
        gpool = ctx.enter_context(tc.tile_pool(name="gath", bufs=2))
        hpool = ctx.enter_context(tc.tile_pool(name="hsex", bufs=2))
        opool = ctx.enter_context(tc.tile_pool(name="oneh", bufs=2))
        zpool = ctx.enter_context(tc.tile_pool(name="zl", bufs=3))
        apool = ctx.enter_context(tc.tile_pool(name="adL", bufs=2))
        epool = ctx.enter_context(tc.tile_pool(name="epi", bufs=3))
        augp = ctx.enter_context(tc.tile_pool(name="augsb", bufs=2))
        psp = ctx.enter_context(tc.tile_pool(name="ps", bufs=3, space="PSUM"))
        pst = ctx.enter_context(tc.tile_pool(name="pst", bufs=2, space="PSUM"))
        psa = ctx.enter_context(tc.tile_pool(name="psa", bufs=2, space="PSUM"))
        psg = ctx.enter_context(tc.tile_pool(name="psg", bufs=1, space="PSUM"))

        # ---- load the packed blob, bitcast apart ----
        xt_sb = cpool.tile([F, NPAD], F16, tag="xt")
        nc.gpsimd.dma_start(out=xt_sb[:],
                            in_=d_m[:, 0:NPAD // 2].bitcast(F8))  # fp8->f16
        b16 = cpool.tile([128, W16], F16, tag="blob16")
        nc.sync.dma_start(out=b16[:], in_=d_m[:, OB16:OB16 + W16].bitcast(F16))
        b32 = cpool.tile([128, N32], F32, tag="blob32")
        nc.sync.dma_start(out=b32[:],
                          in_=d_m[:, OB32:OB32 + 2 * N32].bitcast(F32))
        idxr = cpool.tile([128, NT * 16], I16, tag="idxr")
        nc.sync.dma_start(
            out=idxr[0:16, 0:NT * 8].rearrange("r (j w) -> r j w", j=8),
            in_=d_m[:, OIDX:OIDX + NT].rearrange("(r j) w -> r j w", j=8))

        bias1_sb = bias2_sb = l1b_sb = l2b_sb = None
        if d_bias1 is not None:
            bias1_sb = cpool.tile([128, F], F32, tag="b1")
            nc.sync.dma_start(out=bias1_sb[:], in_=d_bias1[:, :])
        if d_bias2 is not None:
            bias2_sb = cpool.tile([128, F], F32, tag="b2")
            nc.sync.dma_start(out=bias2_sb[:], in_=d_bias2[:, :])
        if d_l1b is not None:
            l1b_sb = cpool.tile([G, C], F32, tag="l1b")
            nc.sync.dma_start(out=l1b_sb[:], in_=d_l1b[:, :])
        if d_l2b is not None:
            l2b_sb = cpool.tile([G, NCLS], F32, tag="l2b")
            nc.sync.dma_start(out=l2b_sb[:], in_=d_l2b[:, :])

        # ---- device-built constants ----
        iota_sb = cpool.tile([128, U * 128], F16, tag="iota")
        nc.gpsimd.iota(out=iota_sb[:], pattern=[[0, U], [1, 128]], base=0,
                       channel_multiplier=0,
                       allow_small_or_imprecise_dtypes=True)
        # identities via two iotas + is_equal (no negative channel mult)
        idh_sb = cpool.tile([128, 128], F16, tag="idh")
        rowh = cpool.tile([128, 128], F16, tag="rowh")
        nc.gpsimd.iota(out=idh_sb[:], pattern=[[1, 128]], base=0,
                       channel_multiplier=0,
                       allow_small_or_imprecise_dtypes=True)
        nc.gpsimd.iota(out=rowh[:], pattern=[[0, 128]], base=0,
                       channel_multiplier=1,
                       allow_small_or_imprecise_dtypes=True)
        nc.vector.tensor_tensor(out=idh_sb[:], in0=idh_sb[:], in1=rowh[:],
                                op=AX.is_equal)
        idf_sb = cpool.tile([64, 64], F32, tag="idf")
        rowf = cpool.tile([64, 64], F32, tag="rowf")
        nc.gpsimd.iota(out=idf_sb[:], pattern=[[1, 64]], base=0,
                       channel_multiplier=0,
                       allow_small_or_imprecise_dtypes=True)
        nc.gpsimd.iota(out=rowf[:], pattern=[[0, 64]], base=0,
                       channel_multiplier=1,
                       allow_small_or_imprecise_dtypes=True)
        nc.vector.tensor_tensor(out=idf_sb[:], in0=idf_sb[:], in1=rowf[:],
                                op=AX.is_equal)
        # graph one-hot: gone[p, t*G+g] = (gid[p,t] == g)
        gone_sb = cpool.tile([128, BLOCKS * G], F16, tag="gone")
        nc.gpsimd.iota(out=gone_sb[:], pattern=[[0, BLOCKS], [1, G]], base=0,
                       channel_multiplier=0,
                       allow_small_or_imprecise_dtypes=True)
        nc.vector.tensor_tensor(
            out=gone_sb[:].rearrange("p (t g) -> p t g", g=G),
            in0=gone_sb[:].rearrange("p (t g) -> p t g", g=G),
            in1=b16[:, OGID:OGID + BLOCKS].to_broadcast([128, BLOCKS, G]),
            op=AX.is_equal)

        # unpack pk -> pdst, psrc, dstl (+ src complement)
        pdst_sb = cpool.tile([128, NT], F16, tag="pdst")
        psrc_sb = cpool.tile([128, NT], F16, tag="psrc")
        dstl_sb = cpool.tile([128, NT], F16, tag="dstl")
        qsrc_sb = cpool.tile([128, NT], F16, tag="qsrc")
        nc.vector.tensor_scalar(out=pdst_sb[:], in0=b16[:, OPK:OPK + NT],
                                scalar1=512.0, scalar2=None, op0=AX.is_ge)
        nc.vector.scalar_tensor_tensor(out=dstl_sb[:], in0=pdst_sb[:],
                                       scalar=-512.0, op0=AX.mult,
                                       in1=b16[:, OPK:OPK + NT], op1=AX.add)
        nc.vector.tensor_scalar(out=psrc_sb[:], in0=dstl_sb[:],
                                scalar1=256.0, scalar2=None, op0=AX.is_ge)
        nc.vector.scalar_tensor_tensor(out=dstl_sb[:], in0=psrc_sb[:],
                                       scalar=-256.0, op0=AX.mult,
                                       in1=dstl_sb[:], op1=AX.add)
        nc.vector.tensor_scalar(out=qsrc_sb[:], in0=psrc_sb[:], scalar1=-1.0,
                                scalar2=1.0, op0=AX.mult, op1=AX.add)

        # ---- derive the dst gather stream on device ----
        # dst pair row = (c*NPAD + dstl*BLOCKS + b - pdst) / 2, computed in
        # f32 (exact), clamped for pad slots, cast to i16 and wrap-shuffled
        # into the [i%16, i//16] stream layout dma_gather expects.
        drow = cpool.tile([128, NT], F32, tag="drow")
        for b in range(BLOCKS):
            nc.vector.memset(drow[:, OFT[b]:OFT[b + 1]], float(b))
        nc.vector.scalar_tensor_tensor(out=drow[:], in0=dstl_sb[:],
                                       scalar=float(BLOCKS), op0=AX.mult,
                                       in1=drow[:], op1=AX.add)
        nc.vector.tensor_scalar(out=drow[:], in0=drow[:],
                                scalar1=b32[:, 27:28], scalar2=None,
                                op0=AX.add)
        nc.vector.scalar_tensor_tensor(out=drow[:], in0=pdst_sb[:],
                                       scalar=-1.0, op0=AX.mult,
                                       in1=drow[:], op1=AX.add)
        nc.vector.tensor_scalar(out=drow[:], in0=drow[:], scalar1=0.5,
                                scalar2=float(TBLROWS // 2 - 1), op0=AX.mult,
                                op1=AX.min)
        ph16 = cpool.tile([128, NT], I16, tag="ph16")
        nc.vector.tensor_copy(out=ph16[:], in_=drow[:])
        wrapv = idxr[0:16, NT * 8:NT * 16].rearrange("r (u q) -> r u q", q=8)
        for q in range(8):
            nc.sync.dma_start(
                out=wrapv[:, :, q:q + 1],
                in_=ph16[q * 16:(q + 1) * 16, :].rearrange(
                    "p (u one) -> p u one", one=1))
        # replicate 16 -> 128 partitions (dma_gather wants the stream on
        # every 16-partition group)
        nc.sync.dma_start(out=idxr[16:32, :], in_=idxr[0:16, :])
        nc.sync.dma_start(out=idxr[32:64, :], in_=idxr[0:32, :])
        nc.sync.dma_start(out=idxr[64:128, :], in_=idxr[0:64, :])

        def build_aug_from_xt(woff):
            """aug rows for own nodes from resident x^T; returns sbuf tile."""
            aug_sb = augp.tile([128, BLOCKS * AUGW], F16, tag="augsb")
            for t in range(BLOCKS):
                ps = psa.tile([128, AUGW], F32, tag="psaug")
                nc.tensor.matmul(out=ps[:], lhsT=xt_sb[:, t * 128:(t + 1) * 128],
                                 rhs=b16[:, woff:woff + AUGW],
                                 start=True, stop=True)
                nc.vector.tensor_copy(out=aug_sb[:, t * AUGW:(t + 1) * AUGW],
                                      in_=ps[:])
            return aug_sb

        def publish_table(aug_sb, which):
            dst = aug_loc[which]
            # DRAM rows r = p*BLOCKS + t  <=> view [(p t), f] -> [p, (t f)]
            nc.sync.dma_start(
                out=dst[:, :].rearrange("(p t) f -> p (t f)", t=BLOCKS),
                in_=aug_sb[:])
            nc.gpsimd.collective_compute(
                "AllGather", AX.bypass, replica_groups=RG,
                ins=[dst[:, :].opt()], outs=[table[which][:, :].opt()])
            # reformat into pair-row gather tables (DRAM->DRAM)
            t3 = table[which][:, :].rearrange("(g two) f -> g two f", two=2)
            nc.sync.dma_start(
                out=hp_tbl[which][:, 0:2 * F].rearrange(
                    "g (two f) -> g two f", two=2),
                in_=t3[:, :, 0:F])
            nc.sync.dma_start(
                out=hp_tbl[which][:, 2 * F:2 * F + 2 * H].rearrange(
                    "g (two a) -> g two a", two=2),
                in_=t3[:, :, F:F + H])
            # full 128-col rows (finite pad): cols 48:64 = a_even,
            # cols 112:128 = a_odd; 0:48/64:112 are h-tail junk
            nc.sync.dma_start(
                out=ap_tbl[which][:, :].rearrange("g (two j) -> g two j", two=2),
                in_=t3[:, :, F - 48:F + 2 * H])

        def elu_inplace(v_sb, width, out_tile):
            """out_tile(fp16) = elu(v_sb) = max(v,0) + min(exp(v)-1, 0)."""
            t_sb = epool.tile([128, width], F32, tag="elu_t")
            nc.scalar.activation(out=t_sb[:], in_=v_sb[:],
                                 func=mybir.ActivationFunctionType.Exp)
            nc.vector.tensor_scalar(out=t_sb[:], in0=t_sb[:], scalar1=1.0,
                                    scalar2=0.0, op0=AX.subtract, op1=AX.min)
            nc.vector.scalar_tensor_tensor(out=out_tile[:], in0=v_sb[:],
                                           scalar=0.0, op0=AX.max,
                                           in1=t_sb[:], op1=AX.add)

        def edge_phase(layer, aug_sb):
            """layer 0: consumes table[0], produces aug tile for table[1].
               layer 1: consumes table[1], accumulates pool psum.  aug_sb is
               the CURRENT layer's local aug tile (for the dense self-loop
               term).  Returns next aug tile (layer 0) or pool psum."""
            bias_sb = (bias1_sb, bias2_sb)[layer]
            if layer == 0:
                out_aug = augp.tile([128, BLOCKS * AUGW], F16, tag="augsb")
            else:
                pool_ps = psg.tile([G, F], F32, tag="poolps")

            hp, ap = hp_tbl[layer], ap_tbl[layer]
            nbatch = (NT + U - 1) // U
            ps_cur = None
            for bi in range(nbatch):
                u0 = bi * U
                ub = min(U, NT - u0)
                # bulk gathers: [h|asrc] pair-rows by src//2 (768B) and a
                # pair-rows by dst//2 (256B) — 2 descriptors per edge
                ghp = gpool.tile([128, U * HPW], F16, tag="g")
                nc.gpsimd.dma_gather(
                    out_ap=ghp[:, :ub * HPW].rearrange(
                        "p (u f) -> p u f", f=HPW),
                    in_ap=hp[:, :], idxs_ap=idxr[:, u0 * 8:(u0 + ub) * 8],
                    num_idxs=ub * 128, num_idxs_reg=ub * 128, elem_size=HPW,
                    single_packet=False)
                gap = apool.tile([128, U * 128], F16, tag="gap")
                nc.gpsimd.dma_gather(
                    out_ap=gap[:, :ub * 128].rearrange(
                        "p (u f) -> p u f", f=128),
                    in_ap=ap[:, :],
                    idxs_ap=idxr[:, NT * 8 + u0 * 8:NT * 8 + (u0 + ub) * 8],
                    num_idxs=ub * 128, num_idxs_reg=ub * 128,
                    elem_size=128, single_packet=False, queue_num=1)
                g3 = ghp[:, :ub * HPW].rearrange("p (u f) -> p u f", f=HPW)
                ga = gap[:, :ub * 128].rearrange("p (u f) -> p u f", f=128)

                # z = asrc[src] + adst[dst] with parity selection:
                #   asrc = ae + psrc*(ao-ae); adst = be + pdst*(bo-be)
                zl = zpool.tile([128, U * H], F16, tag="zl")
                tsel = zpool.tile([128, U * H], F16, tag="tsel")
                psB = psrc_sb[:, u0:u0 + ub].to_broadcast([128, ub, H])
                pdB = pdst_sb[:, u0:u0 + ub].to_broadcast([128, ub, H])
                t3 = tsel[:, :ub * H].rearrange("p (u h) -> p u h", h=H)
                z3 = zl[:, :ub * H].rearrange("p (u h) -> p u h", h=H)
                nc.vector.tensor_tensor(out=t3, in0=g3[:, :, 2 * F + H:2 * F + 2 * H],
                                        in1=g3[:, :, 2 * F:2 * F + H],
                                        op=AX.subtract)
                nc.vector.tensor_tensor(out=t3, in0=t3, in1=psB, op=AX.mult)
                nc.vector.tensor_tensor(out=z3, in0=t3,
                                        in1=g3[:, :, 2 * F:2 * F + H], op=AX.add)
                nc.vector.tensor_tensor(out=t3, in0=ga[:, :, 120:128],
                                        in1=ga[:, :, 56:64], op=AX.subtract)
                nc.vector.tensor_tensor(out=t3, in0=t3, in1=pdB, op=AX.mult)
                nc.vector.tensor_tensor(out=z3, in0=z3, in1=t3, op=AX.add)
                nc.vector.tensor_tensor(out=z3, in0=z3,
                                        in1=ga[:, :, 56:64], op=AX.add)
                zv = zl[:, :ub * H]
                nc.vector.scalar_tensor_tensor(
                    out=zv, in0=zv, scalar=0.2, op0=AX.mult, in1=zv, op1=AX.max)

                he = hpool.tile([128, U * REPW], F16, tag="he")
                he3 = he[:, :ub * REPW].rearrange("p (u f) -> p u f", f=REPW)
                nc.scalar.activation(
                    out=he3[:, :, 2 * F:2 * F + H],
                    in_=zl[:, :ub * H].rearrange("p (u h) -> p u h", h=H),
                    func=mybir.ActivationFunctionType.Exp)
                # parity-masked ex, folded into the h scaling: the even half is
                # scaled by ex*(1-psrc), the odd half by ex*psrc, so the wrong
                # parity contributes zero and the psum halves sum to the answer
                exE = zpool.tile([128, U * H], F16, tag="exE")
                exO = zpool.tile([128, U * H], F16, tag="exO")
                eE3 = exE[:, :ub * H].rearrange("p (u h) -> p u h", h=H)
                eO3 = exO[:, :ub * H].rearrange("p (u h) -> p u h", h=H)
                nc.vector.tensor_tensor(
                    out=eE3, in0=he3[:, :, 2 * F:2 * F + H],
                    in1=qsrc_sb[:, u0:u0 + ub].to_broadcast([128, ub, H]),
                    op=AX.mult)
                nc.vector.tensor_tensor(
                    out=eO3, in0=he3[:, :, 2 * F:2 * F + H], in1=psB,
                    op=AX.mult)
                nc.vector.tensor_tensor(
                    out=he3[:, :, 0:F].rearrange("p u (h c) -> p u h c", c=C),
                    in0=g3[:, :, 0:F].rearrange("p u (h c) -> p u h c", c=C),
                    in1=eE3.to_broadcast([128, ub, H, C]), op=AX.mult)
                nc.vector.tensor_tensor(
                    out=he3[:, :, F:2 * F].rearrange("p u (h c) -> p u h c", c=C),
                    in0=g3[:, :, F:2 * F].rearrange("p u (h c) -> p u h c", c=C),
                    in1=eO3.to_broadcast([128, ub, H, C]), op=AX.mult)

                oh = opool.tile([128, U * 128], F16, tag="oh")
                nc.vector.tensor_tensor(
                    out=oh[:, :ub * 128].rearrange("p (u j) -> p u j", j=128),
                    in0=iota_sb[:, :ub * 128].rearrange("p (u j) -> p u j", j=128),
                    in1=dstl_sb[:, u0:u0 + ub].to_broadcast([128, ub, 128]),
                    op=AX.is_equal)

                for u in range(ub):
                    t = u0 + u
                    b, k = tilemap[t]
                    if k == 0:
                        ps_cur = psp.tile([128, REPW], F32, tag="psblk")
                    nc.tensor.matmul(
                        out=ps_cur[:], lhsT=oh[:, u * 128:(u + 1) * 128],
                        rhs=he[:, u * REPW:(u + 1) * REPW],
                        start=(k == 0), stop=(k == TBS[b] - 1))
                    if k == TBS[b] - 1:
                        # ---- block epilogue ----
                        # dense self-loop term from the local aug rows:
                        # ex_self = exp(leaky(asrc+adst)), s += ex_self,
                        # num += h_local * ex_self
                        zs = epool.tile([128, H], F16, tag="zs")
                        nc.vector.tensor_tensor(
                            out=zs[:], in0=aug_sb[:, b * AUGW + F:b * AUGW + F + H],
                            in1=aug_sb[:, b * AUGW + F + H:b * AUGW + F + 2 * H],
                            op=AX.add)
                        nc.vector.scalar_tensor_tensor(
                            out=zs[:], in0=zs[:], scalar=0.2, op0=AX.mult,
                            in1=zs[:], op1=AX.max)
                        exs = epool.tile([128, H], F32, tag="exs")
                        nc.scalar.activation(out=exs[:], in_=zs[:],
                                             func=mybir.ActivationFunctionType.Exp)
                        s_sb = epool.tile([128, H], F32, tag="s")
                        nc.vector.tensor_tensor(out=s_sb[:], in0=exs[:],
                                                in1=ps_cur[:, 2 * F:2 * F + H],
                                                op=AX.add)
                        nc.vector.tensor_scalar(out=s_sb[:], in0=s_sb[:],
                                                scalar1=1e-30, scalar2=None,
                                                op0=AX.max)
                        r_sb = epool.tile([128, H], F32, tag="r")
                        nc.vector.reciprocal(out=r_sb[:], in_=s_sb[:])
                        hs_sb = epool.tile([128, F], F32, tag="hs")
                        nc.vector.tensor_tensor(
                            out=hs_sb[:].rearrange("p (h c) -> p h c", c=C),
                            in0=aug_sb[:, b * AUGW:b * AUGW + F].rearrange(
                                "p (h c) -> p h c", c=C),
                            in1=exs[:].to_broadcast([128, H, C]), op=AX.mult)
                        hc_sb = epool.tile([128, F], F32, tag="hc")
                        nc.vector.tensor_tensor(out=hc_sb[:], in0=hs_sb[:],
                                                in1=ps_cur[:, 0:F], op=AX.add)
                        nc.vector.tensor_tensor(out=hc_sb[:], in0=hc_sb[:],
                                                in1=ps_cur[:, F:2 * F], op=AX.add)
                        v_sb = epool.tile([128, F], F32, tag="v")
                        nc.vector.tensor_tensor(
                            out=v_sb[:].rearrange("p (h c) -> p h c", c=C),
                            in0=hc_sb[:].rearrange("p (h c) -> p h c", c=C),
                            in1=r_sb[:].to_broadcast([128, H, C]), op=AX.mult)
                        if bias_sb is not None:
                            nc.vector.tensor_tensor(out=v_sb[:], in0=v_sb[:],
                                                    in1=bias_sb[:], op=AX.add)
                        eo = epool.tile([128, F], F16, tag="eo")
                        elu_inplace(v_sb, F, eo)
                        if layer == 0:
                            trp = pst.tile([128, 128], F16, tag="trps")
                            nc.tensor.transpose(out=trp[:], in_=eo[:],
                                                identity=idh_sb[:])
                            trs = epool.tile([128, 128], F16, tag="trsb")
                            nc.vector.tensor_copy(out=trs[:], in_=trp[:])
                            ap2 = psa.tile([128, AUGW], F32, tag="psaug")
                            nc.tensor.matmul(out=ap2[:], lhsT=trs[:],
                                             rhs=b16[:, OW2:OW2 + AUGW],
                                             start=True, stop=True)
                            nc.vector.tensor_copy(
                                out=out_aug[:, b * AUGW:(b + 1) * AUGW],
                                in_=ap2[:])
                        else:
                            nc.tensor.matmul(
                                out=pool_ps[:],
                                lhsT=gone_sb[:, b * G:(b + 1) * G],
                                rhs=eo[:], start=(b == 0), stop=(b == BLOCKS - 1))
            return out_aug if layer == 0 else pool_ps

        # ---------------- pipeline ----------------
        aug1_sb = build_aug_from_xt(OW1)
        publish_table(aug1_sb, 0)
        aug2_sb = edge_phase(0, aug1_sb)
        publish_table(aug2_sb, 1)
        pool_ps = edge_phase(1, aug2_sb)

        # pooling allreduce
        psum_sb = epool.tile([G, F], F32, tag="poolsb")
        nc.vector.tensor_copy(out=psum_sb[:], in_=pool_ps[:])
        nc.sync.dma_start(out=pool_part[:, :], in_=psum_sb[:])
        nc.gpsimd.collective_compute(
            "AllReduce", AX.add, replica_groups=RG,
            ins=[pool_part[:, :].opt()], outs=[pool_full[:, :].opt()])
        hg_sb = epool.tile([G, F], F32, tag="hg")
        nc.sync.dma_start(out=hg_sb[:], in_=pool_full[:, :])
        nc.vector.tensor_scalar(out=hg_sb[:], in0=hg_sb[:],
                                scalar1=b32[0:G, 16:17], scalar2=None,
                                op0=AX.mult)

        # MLP: z1 = elu(hg @ lin1W + b); logits = z1 @ lin2W + b
        hgT_ps = pst.tile([F, G], F32, tag="trps")
        nc.tensor.transpose(out=hgT_ps[:], in_=hg_sb[:], identity=idf_sb[:G, :G])
        hgT_sb = epool.tile([F, G], F32, tag="hgTs")
        nc.vector.tensor_copy(out=hgT_sb[:], in_=hgT_ps[:])
        z1_ps = psa.tile([G, C], F32, tag="psaug")
        nc.tensor.matmul(out=z1_ps[:], lhsT=hgT_sb[:], rhs=b32[:, 0:16],
                         start=True, stop=True)
        z1_sb = epool.tile([G, C], F32, tag="z1s")
        if l1b_sb is not None:
            nc.vector.tensor_tensor(out=z1_sb[:], in0=z1_ps[:], in1=l1b_sb[:],
                                    op=AX.add)
        else:
            nc.vector.tensor_copy(out=z1_sb[:], in_=z1_ps[:])
        z1e_sb = epool.tile([G, C], F32, tag="z1e")
        t1 = epool.tile([G, C], F32, tag="t1")
        nc.scalar.activation(out=t1[:], in_=z1_sb[:],
                             func=mybir.ActivationFunctionType.Exp)
        nc.vector.tensor_scalar(out=t1[:], in0=t1[:], scalar1=1.0, scalar2=0.0,
                                op0=AX.subtract, op1=AX.min)
        nc.vector.scalar_tensor_tensor(out=z1e_sb[:], in0=z1_sb[:], scalar=0.0,
                                       op0=AX.max, in1=t1[:], op1=AX.add)
        z1T_ps = pst.tile([C, G], F32, tag="trps")
        nc.tensor.transpose(out=z1T_ps[:], in_=z1e_sb[:], identity=idf_sb[:G, :G])
        z1T_sb = epool.tile([C, G], F32, tag="z1Ts")
        nc.vector.tensor_copy(out=z1T_sb[:], in_=z1T_ps[:])
        lg_ps = psa.tile([G, NCLS], F32, tag="psaug")
        nc.tensor.matmul(out=lg_ps[:], lhsT=z1T_sb[:], rhs=b32[0:16, 17:17 + NCLS],
                         start=True, stop=True)
        lg_sb = epool.tile([G, NCLS], F32, tag="lgs")
        if l2b_sb is not None:
            nc.vector.tensor_tensor(out=lg_sb[:], in0=lg_ps[:], in1=l2b_sb[:],
                                    op=AX.add)
        else:
            nc.vector.tensor_copy(out=lg_sb[:], in_=lg_ps[:])

        # log_softmax
        m_sb = epool.tile([G, 1], F32, tag="m")
        nc.vector.tensor_reduce(out=m_sb[:], in_=lg_sb[:],
                                axis=mybir.AxisListType.X, op=AX.max)
        nm_sb = epool.tile([G, 1], F32, tag="nm")
        nc.vector.tensor_scalar(out=nm_sb[:], in0=m_sb[:], scalar1=-1.0,
                                scalar2=None, op0=AX.mult)
        e_sb = epool.tile([G, NCLS], F32, tag="esm")
        ss_sb = epool.tile([G, 1], F32, tag="ss")
        nc.scalar.activation(out=e_sb[:], in_=lg_sb[:],
                             func=mybir.ActivationFunctionType.Exp,
                             bias=nm_sb[:, 0:1], accum_out=ss_sb[:, 0:1])
        ls_sb = epool.tile([G, 1], F32, tag="ls")
        nc.scalar.activation(out=ls_sb[:], in_=ss_sb[:],
                             func=mybir.ActivationFunctionType.Ln)
        lsm_sb = epool.tile([G, NCLS], F32, tag="lsm")
        nc.vector.tensor_scalar(out=lsm_sb[:], in0=lg_sb[:],
                                scalar1=m_sb[:, 0:1], scalar2=ls_sb[:, 0:1],
                                op0=AX.subtract, op1=AX.subtract)

        nc.sync.dma_start(out=d_out[0:G, :], in_=lsm_sb[:])
        nc.sync.dma_start(out=d_out[G:2 * G, :], in_=lg_sb[:])

    nc.compile()  # bacc register allocation / DCE / act-table loads
    # The module is immutable from here on; memoize its (deterministic)
    # serialization so each run_bass_kernel_spmd call skips the ~0.1s
    # re-serialization in the jit lowering path.
    _json = nc.to_json_bytes()
    nc.to_json_bytes = lambda: _json
    return nc


def run_gat(inputs, cfg, trace=False):
    meta, in_maps = host_prep(inputs, cfg)
    nc = build_nc(meta)
    res = run_bass_kernel_spmd(nc, in_maps, core_ids=list(range(NCORES)),
                               trace=trace)
    G, NCLS = cfg["G"], cfg["NCLS"]
    out = res.results[0]["out"]
    return (out[0:G, :], out[G:2 * G, :]), res


def kernel(**inputs):
    (lsm, logits), _ = run_gat(inputs, gat_config())
    return lsm.astype(np.float32), logits.astype(np.float32)
